# revision 30
# baseline (speedup 1.0000x reference)
"""Multi-head self-attention (RoPE, causal) Trainium2 Bass kernel.

Problem: B=4, S=2048, D=1024, H=16 heads, d_k=64, f32 in/out.

Sharding: head-parallel across 8 NeuronCores. Core c owns heads {2c, 2c+1}
and all batches/tokens. QKV projections are column-parallel (each core
computes only its heads' features), attention is fully local per core, and
the output projection is column-parallel after a per-batch AllGather of the
per-core attention outputs (each core computes 128 of the 1024 output
features).

Layouts (transposed activations, [feature, token]):
  - host pre-transposes x to xT [D, B*S]
  - Q/K projections produce per-head parity-split rows
    [h0ev(32), h0od(32), h1ev(32), h1od(32)]; rope is
    QR = xq*C + swap32(xq)*D with C = [cos]*4, D = [-sin, sin, -sin, sin]
    (swap32 exchanges each adjacent 32-row pair via 4 SBUF->SBUF DMAs);
    scores q.k are invariant under the shared q/k dim permutation
  - q-rope runs on DVE, k-rope on GPSIMD (Pool); q-swaps on SP HWDGE,
    k-swaps on Pool SWDGE to spread DGE cost
  - scores are computed transposed, S^T [k-partitions, q-free], with the
    two heads' matmuls packed as concurrent 64x128 PE row-tiles
    (tile_position (0,0)/(64,0)) writing separate PSUM banks; one exp
    activation covers both heads per k-block
  - attention is phase-separated per q-chunk: all score blocks + exp into
    SBUF pt tiles first, then both heads' P@V accumulation chains (fewer
    PE tile-mode switches, uninterrupted PSUM accumulation)
  - V is projected transposed then PE-transposed to [token, feature] with a
    per-head ones column so each P@V matmul also emits the softmax
    denominator as output row 64; the denominator row is DMA'd straight
    from PSUM
  - normalization is batched: denominator rows gathered per batch, one wide
    reciprocal, selector-matmul broadcast to [128, QC] and one in-place
    multiply per token chunk
"""

import numpy as np
import ml_dtypes
from contextlib import ExitStack

import concourse.bass as bass
import concourse.bacc as bacc
import concourse.tile as tile
from concourse import mybir
from concourse.bass_utils import run_bass_kernel_spmd
from concourse.masks import make_upper_triangular, make_identity

F32 = mybir.dt.float32
BF16 = mybir.dt.bfloat16

B, S, D, H = 4, 2048, 1024, 16
NC = 8
DK = 64
HPC = H // NC  # heads per core
THETA = 100000.0
QC = 512       # q chunk (tokens per score-strip column block)
KB = 128       # k block (tokens per score-strip partition block)
NDB = D // 128  # number of 128-wide contraction blocks

ts = bass.ts
ds = bass.ds


def build_program(seq=S, mm_dt=BF16, n_cores=NC, phases=3,
                  collective=True, reps=1):
    """Build the SPMD Bass/Tile program (identical on every core).

    collective=False replaces the AllGather with a same-shape local DMA
    (for single-core TimelineSim timing analysis only).
    reps>1 repeats the whole kernel body back-to-back inside one NEFF
    (for on-hardware marginal-time benchmarking).
    """
    nc = bacc.Bacc("TRN2", target_bir_lowering=False, debug=False,
                   num_devices=n_cores)
    T = B * seq
    nqc = seq // QC
    nkb = seq // KB
    ntb = T // KB

    xT_h = nc.declare_dram_parameter("xT", [D, T], mm_dt, isOutput=False)
    wqk_h = nc.declare_dram_parameter("wqkT", [128, 2, NDB, 128], mm_dt,
                                      isOutput=False)
    wv_h = nc.declare_dram_parameter("wvT", [128, NDB, 128], mm_dt, isOutput=False)
    wo_h = nc.declare_dram_parameter("woT", [128, NDB, 128], mm_dt, isOutput=False)
    ra_h = nc.declare_dram_parameter("ropeA", [128, seq], mm_dt, isOutput=False)
    rb_h = nc.declare_dram_parameter("ropeB", [128, seq], mm_dt, isOutput=False)
    sel_h = nc.declare_dram_parameter("selT", [HPC, 128],
                                      mybir.dt.float32r, isOutput=False)
    yT_h = nc.declare_dram_parameter("yT", [128, T], F32, isOutput=True)

    EXP = mybir.ActivationFunctionType.Exp
    scale = 1.0 / float(np.sqrt(DK))
    KPQ = QC // KB  # k-blocks per q-chunk

    with tile.TileContext(nc, num_cores=n_cores) as tc, ExitStack() as ctx:
        consts = ctx.enter_context(tc.tile_pool(name="consts", bufs=1))
        persist = ctx.enter_context(tc.tile_pool(name="persist", bufs=1))

        ra_t = consts.tile([128, seq], mm_dt)
        nc.sync.dma_start(ra_t[:], ra_h[:, :])
        rb_t = consts.tile([128, seq], mm_dt)
        nc.sync.dma_start(rb_t[:], rb_h[:, :])
        wqks = consts.tile([128, 2, NDB, 128], mm_dt)
        nc.sync.dma_start(wqks[:], wqk_h[:, :, :, :])
        wvs = consts.tile([128, NDB, 128], mm_dt)
        nc.sync.dma_start(wvs[:], wv_h[:, :, :])
        wos = consts.tile([128, NDB, 128], mm_dt)
        nc.sync.dma_start(wos[:], wo_h[:, :, :])
        sels = consts.tile([HPC, 128], mybir.dt.float32r)
        nc.sync.dma_start(sels[:], sel_h[:, :])
        triu = consts.tile([128, 128], mm_dt)
        make_upper_triangular(nc, triu[:], val=1.0, diag=True)
        ident = consts.tile([128, 128], mm_dt)
        make_identity(nc, ident[:])

        # QR/KR rows: [h0_r1(0:32), h0_r2(32:64), h1_r1(64:96), h1_r2(96:128)]
        QR = persist.tile([128, T], mm_dt)
        KR = persist.tile([128, T], mm_dt)
        # V in [token, feature] layout per 128-token block; per head a
        # 65-col block [v_h(0:64) | ones(64)] so P@V emits the softmax
        # denominator as out row 64 for either head
        VA = persist.tile([128, ntb, 130], mm_dt)
        # attention output slice, transposed: rows [h0 feats(0:64), h1(64:128)]
        ATT = persist.tile([128, T], mm_dt)

        # ---- Interleaved per-batch pipeline ------------------------------
        with (tc.tile_pool(name="ph1x", bufs=2) as xpool,
              tc.tile_pool(name="ph1s", bufs=2) as sbp,
              tc.tile_pool(name="dupt", bufs=2) as dpup,
              tc.tile_pool(name="ropet", bufs=2) as rpool,
              tc.tile_pool(name="sps", bufs=2, space="PSUM") as sps,
              tc.tile_pool(name="pvps", bufs=2, space="PSUM") as pvps,
              tc.tile_pool(name="auxps", bufs=2, space="PSUM") as aux,
              tc.tile_pool(name="ptp", bufs=18) as ptp,
              tc.tile_pool(name="nrm", bufs=2) as nrm,
              tc.tile_pool(name="dram", bufs=1, space="DRAM") as dpool,
              tc.tile_pool(name="agp", bufs=2) as agp,
              tc.tile_pool(name="ysb", bufs=3) as ysb):
            do_p3 = phases >= 3
            ag_space = "Shared" if collective else "Local"
            nc.vector.memset(VA[:, :, 64], 1.0)
            nc.vector.memset(VA[:, :, 129], 1.0)
            xr = xT_h[:, :].rearrange("(i p) t -> p i t", p=128)

            def all_gather(inp, outp):
                if collective:
                    nc.gpsimd.collective_compute(
                        "AllGather", mybir.AluOpType.bypass,
                        replica_groups=[list(range(n_cores))],
                        ins=[inp[:, :].opt()], outs=[outp[:, :].opt()])
                else:  # sim stand-in for the SDMA traffic (SP queue)
                    for cc in range(n_cores):
                        nc.sync.dma_start(
                            outp[ds(128 * cc, 128), :], inp[:, :])

            def wo_chunk(agc, bb, cc):
                agr = agc[:, :].rearrange("(i p) t -> p i t", p=128)
                rt = agp.tile([128, NDB, QC], mm_dt, tag="rt", name="rt")
                nc.sync.dma_start(rt[:], agr[:, :, :])
                yp = aux.tile([128, QC], F32, tag="aux", name="yp")
                for i in range(NDB):
                    nc.tensor.matmul(yp[:], wos[:, i, :], rt[:, i, :],
                                     start=(i == 0), stop=(i == NDB - 1))
                ysb_t = ysb.tile([128, QC], F32, tag="ys", name="ysb_t")
                nc.vector.tensor_copy(ysb_t[:], yp[:])
                nc.sync.dma_start(yT_h[:, ds(bb * seq + cc * QC, QC)],
                                  ysb_t[:])

            for rep in range(reps):
              # per-(batch, chunk) AllGather buffers: spreads collective +
              # HBM traffic through the batch (fresh set per rep: Shared
              # tiles allow only one writer)
              attcs, agcs = [], []
              for _b in range(B):
                attcs.append([dpool.tile([128, QC], mm_dt,
                                         name=f"attc{rep}_{_b}_{_q}")
                              for _q in range(nqc)])
                agcs.append([dpool.tile([128 * n_cores, QC], mm_dt,
                                        addr_space=ag_space,
                                        name=f"agc{rep}_{_b}_{_q}")
                             for _q in range(nqc)])
              for b in range(B):
                # ---- projections + swap-DMA + rope + V for 4 chunks ----
                for ci in range(nqc):
                    c = b * nqc + ci
                    bsl = ds(ci * QC, QC)
                    xc = xpool.tile([128, NDB, QC], mm_dt, tag="xc")
                    # ACT's DGE queue: keeps batch-tail SP traffic (AG/rt/
                    # stand-ins) from delaying the next batch's x loads
                    nc.scalar.dma_start(xc[:], xr[:, :, ts(c, QC)])

                    xsb = []
                    for g in range(2):
                        ps = aux.tile([128, QC], F32, tag="aux", name="ps")
                        for i in range(NDB):
                            nc.tensor.matmul(ps[:], wqks[:, g, i, :],
                                             xc[:, i, :],
                                             start=(i == 0), stop=(i == NDB - 1))
                        xg = sbp.tile([128, QC], mm_dt, tag=f"xg{g}", name="xg")
                        nc.vector.tensor_copy(xg[:], ps[:])
                        xsb.append(xg)
                    # swap32: exchange adjacent 32-row pairs so x2 sits under
                    # x1's lanes (and vice versa); q on SP HWDGE, k on ACT
                    # HWDGE queue
                    sx = []
                    for g, eng in ((0, nc.sync), (1, nc.scalar)):
                        sxt = dpup.tile([128, QC], mm_dt, tag=f"sx{g}",
                                        name="sx")
                        for blk in range(2):
                            for du in range(2):
                                eng.dma_start(
                                    sxt[ds(64 * blk + 32 * du, 32), :],
                                    xsb[g][ds(64 * blk + 32 * (1 - du), 32), :])
                        sx.append(sxt)
                    # rope: QR/KR = x*C + swap32(x)*D  (q on DVE, k on Pool)
                    for g, OUT, eng in ((0, QR, nc.vector),
                                        (1, KR, nc.gpsimd)):
                        t1 = rpool.tile([128, QC], mm_dt, tag=f"t1{g}",
                                        name="t1")
                        t2 = rpool.tile([128, QC], mm_dt, tag=f"t2{g}",
                                        name="t2")
                        eng.tensor_mul(t1[:], xsb[g][:], ra_t[:, bsl])
                        eng.tensor_mul(t2[:], sx[g][:], rb_t[:, bsl])
                        eng.tensor_add(OUT[:, ts(c, QC)], t1[:], t2[:])

                    # V: project transposed then PE-transpose 128-blocks
                    psv = aux.tile([128, QC], F32, tag="aux", name="psv")
                    for i in range(NDB):
                        nc.tensor.matmul(psv[:], wvs[:, i, :], xc[:, i, :],
                                         start=(i == 0), stop=(i == NDB - 1))
                    vtc = sbp.tile([128, QC], mm_dt, tag="vtc", name="vtc")
                    nc.vector.tensor_copy(vtc[:], psv[:])
                    for sb in range(QC // 128):
                        tb = c * (QC // 128) + sb
                        tp = aux.tile([128, 128], mm_dt, tag="aux", name="tp")
                        nc.tensor.transpose(tp[:], vtc[:, ts(sb, 128)],
                                            ident[:])
                        vav = VA[:, tb, 0:130].rearrange(
                            "p (a c) -> p a c", a=2, c=65)[:, :, 0:64]
                        tpv = tp[:, :].rearrange("p (a c) -> p a c",
                                                 a=2, c=64)
                        nc.vector.tensor_copy(vav, tpv)

                last = b == B - 1
                # ---- attention for this batch (per q-chunk, two phases) --
                for qi in range(nqc):
                    qsl = ds(b * seq + qi * QC, QC)
                    nk = (qi + 1) * KPQ
                    # phase 1: scores (row-tiled head pair) + exp -> pt
                    pts = []
                    for kb in range(nk):
                        dj = kb - qi * KPQ
                        qv = 128 * dj if dj > 0 else 0
                        ksl = ds(b * seq + kb * KB, KB)
                        qslv = ds(b * seq + qi * QC + qv, QC - qv)
                        sp = sps.tile([128, 2, QC], F32, tag="sp", name="sp")
                        for hl in range(HPC):
                            nc.tensor.matmul(sp[:, hl, qv:QC],
                                             KR[ds(64 * hl, 64), ksl],
                                             QR[ds(64 * hl, 64), qslv],
                                             start=True, stop=True,
                                             tile_position=(64 * hl, 0))
                        pt = ptp.tile([128, 2, QC], mm_dt, tag="pt",
                                      name="pt")
                        nc.scalar.activation(pt[:, :, qv:QC], sp[:, :, qv:QC],
                                             EXP, scale=scale)
                        if dj >= 0:  # diagonal block: causal mask both heads
                            dsl = ds(128 * dj, 128)
                            for hl, eng in ((0, nc.vector), (1, nc.gpsimd)):
                                eng.tensor_mul(pt[:, hl, dsl],
                                               pt[:, hl, dsl],
                                               triu[:])
                        pts.append((pt, qv))
                    # phase 2: P@V accumulation per head + denominator row
                    dbt = nrm.tile([HPC, QC], F32, tag="dbt", name="dbt")
                    for hl in range(HPC):
                        pv = pvps.tile([65, QC], F32, tag="pv", name="pv")
                        for kb in range(nk):
                            pt, qv = pts[kb]
                            tbg = b * nkb + kb
                            nc.tensor.matmul(pv[:, qv:QC],
                                             VA[:, tbg, ds(65 * hl, 65)],
                                             pt[:, hl, qv:QC],
                                             start=(kb == 0),
                                             stop=(kb == nk - 1))
                        nc.vector.tensor_copy(ATT[ds(64 * hl, 64), qsl],
                                              pv[ds(0, 64), :])
                        deng = nrm.tile([65, QC], F32, tag="deng",
                                        name="deng")
                        nc.vector.tensor_copy(deng[ds(64, 1), :],
                                              pv[ds(64, 1), :])
                        nc.sync.dma_start(dbt[ds(hl, 1), :],
                                          deng[ds(64, 1), :])
                    # ---- per-chunk normalize ----
                    rbt = nrm.tile([HPC, QC], mybir.dt.float32r,
                                   tag="rbt", name="rbt")
                    with nc.allow_low_precision(reason="fp32r bcast rhs"):
                        nc.vector.reciprocal(rbt[:], dbt[:])
                    bp = aux.tile([128, QC], F32, tag="aux", name="bp")
                    nc.tensor.matmul(bp[:], sels[:, :], rbt[:],
                                     start=True, stop=True)
                    nc.vector.tensor_mul(ATT[:, qsl], ATT[:, qsl], bp[:])

                    if do_p3:
                        # chunk AllGather; w_o chunk-deferred one batch
                        # (same-batch for the last batch's tail pipeline)
                        # keep Pool's queue free: it blocks on collective
                        # completion waits, so nothing else may queue there
                        nc.sync.dma_start(attcs[b][qi][:, :], ATT[:, qsl])
                        if b > 0:
                            wo_chunk(agcs[b - 1][qi], b - 1, qi)
                        all_gather(attcs[b][qi], agcs[b][qi])
                        if last:
                            wo_chunk(agcs[b][qi], b, qi)
            if not do_p3:
                nc.sync.dma_start(yT_h[:, 0:seq], ra_t[:, :])

    nc.compile()
    return nc


def prep_inputs(inputs, seq=S, mm_dt=BF16, n_cores=NC):
    """Host-side sharding: build the per-core input maps."""
    mm_np = ml_dtypes.bfloat16 if mm_dt == BF16 else np.float32
    x = np.asarray(inputs["in_features"], dtype=np.float32)
    pos = np.asarray(inputs["token_positions"]).astype(np.float32)
    wq = np.asarray(inputs["w_q"], dtype=np.float32)
    wk = np.asarray(inputs["w_k"], dtype=np.float32)
    wv = np.asarray(inputs["w_v"], dtype=np.float32)
    wo = np.asarray(inputs["w_o"], dtype=np.float32)

    T = B * seq
    xT = np.ascontiguousarray(x.reshape(T, D).T).astype(mm_np)

    # rope tables (f32, matching reference numerics):
    #   QR = xq*C + swap32(xq)*D with xq rows [h0ev, h0od, h1ev, h1od];
    #   C = [cos]*4 ; D = [-sin, sin, -sin, sin]
    inv = np.float32(THETA) ** (-np.arange(0, DK, 2, dtype=np.float32)
                                / np.float32(DK))
    ang = pos[:, None].astype(np.float32) * inv[None, :].astype(np.float32)
    cosT = np.cos(ang.astype(np.float32)).T  # [32, seq]
    sinT = np.sin(ang.astype(np.float32)).T
    ropeA = np.ascontiguousarray(
        np.concatenate([cosT, cosT, cosT, cosT], axis=0)).astype(mm_np)
    ropeB = np.ascontiguousarray(
        np.concatenate([-sinT, sinT, -sinT, sinT], axis=0)).astype(mm_np)

    # normalize broadcast selectors: out row m takes denominator row m // 64
    selT = np.zeros((HPC, 128), dtype=np.float32)
    for m in range(128):
        selT[m // 64, m] = 1.0

    ev = np.arange(0, DK, 2)
    od = ev + 1

    def lhsT_stack(W):
        # W [128 out-features, D] -> [128, NDB, 128]; [:, i, :] = W[:, 128i:+128].T
        Wt = np.ascontiguousarray(W.T).astype(mm_np)  # [D, 128]
        return np.ascontiguousarray(
            Wt.reshape(NDB, 128, 128).transpose(1, 0, 2))

    in_maps = []
    for c in range(n_cores):
        h0, h1 = HPC * c, HPC * c + 1
        W1 = np.concatenate([wq[DK * h0 + ev], wq[DK * h0 + od],
                             wq[DK * h1 + ev], wq[DK * h1 + od]], axis=0)
        W2 = np.concatenate([wk[DK * h0 + ev], wk[DK * h0 + od],
                             wk[DK * h1 + ev], wk[DK * h1 + od]], axis=0)
        wqkT = np.ascontiguousarray(np.stack(
            [lhsT_stack(Wg) for Wg in (W1, W2)], axis=1))
        WV = wv[128 * c: 128 * (c + 1)]
        WO = wo[128 * c: 128 * (c + 1)]
        in_maps.append({
            "xT": xT,
            "wqkT": wqkT,
            "wvT": lhsT_stack(WV),
            "woT": lhsT_stack(WO),
            "ropeA": ropeA,
            "ropeB": ropeB,
            "selT": selT,
        })
    return in_maps


def assemble_output(results, seq=S, n_cores=NC):
    yT = np.concatenate([np.asarray(r["yT"], dtype=np.float32)
                         for r in results], axis=0)  # [1024, T]
    return np.ascontiguousarray(yT.T).reshape(B, seq, D).astype(np.float32)


_PROGRAM_CACHE = {}


def kernel(**inputs) -> np.ndarray:
    key = ("full", S, "bf16")
    if key not in _PROGRAM_CACHE:
        _PROGRAM_CACHE[key] = build_program(seq=S, mm_dt=BF16, n_cores=NC)
    nc = _PROGRAM_CACHE[key]
    in_maps = prep_inputs(inputs, seq=S, mm_dt=BF16, n_cores=NC)
    res = run_bass_kernel_spmd(nc, in_maps, list(range(NC)))
    return assemble_output(res.results)


if __name__ == "__main__":
    from ref_np import make_inputs
    inputs = make_inputs(seed=0, S=S)
    out = kernel(**inputs)
    print(out.shape, out.dtype)


# revision 35
# speedup vs baseline: 1.5564x; 1.5564x over previous
"""Multi-head self-attention (RoPE, causal) Trainium2 Bass kernel.

Problem: B=4, S=2048, D=1024, H=16 heads, d_k=64, f32 in/out.

Sharding: head-parallel across 8 NeuronCores. Core c owns heads {2c, 2c+1}
and all batches/tokens. QKV projections are column-parallel (each core
computes only its heads' features), attention is fully local per core, and
the output projection is column-parallel after a per-batch AllGather of the
per-core attention outputs (each core computes 128 of the 1024 output
features).

Layouts (transposed activations, [feature, token]):
  - host pre-transposes x to xT [D, B*S]
  - Q/K projections produce per-head parity-split rows
    [h0ev(32), h0od(32), h1ev(32), h1od(32)]; rope is
    QR = xq*C + swap32(xq)*D with C = [cos]*4, D = [-sin, sin, -sin, sin]
    (swap32 exchanges each adjacent 32-row pair via 4 SBUF->SBUF DMAs);
    scores q.k are invariant under the shared q/k dim permutation
  - q-rope runs on DVE, k-rope on GPSIMD (Pool); q-swaps on SP HWDGE,
    k-swaps on Pool SWDGE to spread DGE cost
  - scores are computed transposed, S^T [k-partitions, q-free], with the
    two heads' matmuls packed as concurrent 64x128 PE row-tiles
    (tile_position (0,0)/(64,0)) writing separate PSUM banks; one exp
    activation covers both heads per k-block
  - attention is phase-separated per q-chunk: all score blocks + exp into
    SBUF pt tiles first, then both heads' P@V accumulation chains (fewer
    PE tile-mode switches, uninterrupted PSUM accumulation)
  - V is projected transposed then PE-transposed to [token, feature] with a
    per-head ones column so each P@V matmul also emits the softmax
    denominator as output row 64; the denominator row is DMA'd straight
    from PSUM
  - normalization is batched: denominator rows gathered per batch, one wide
    reciprocal, selector-matmul broadcast to [128, QC] and one in-place
    multiply per token chunk
"""

import numpy as np
import ml_dtypes
from contextlib import ExitStack

import concourse.bass as bass
import concourse.bacc as bacc
import concourse.tile as tile
from concourse import mybir
from concourse.bass_utils import run_bass_kernel_spmd
from concourse.masks import make_upper_triangular, make_identity

F32 = mybir.dt.float32
BF16 = mybir.dt.bfloat16

B, S, D, H = 4, 2048, 1024, 16
NC = 8
DK = 64
HPC = H // NC  # heads per core
THETA = 100000.0
QC = 512       # q chunk (tokens per score-strip column block)
KB = 128       # k block (tokens per score-strip partition block)
NDB = D // 128  # number of 128-wide contraction blocks

ts = bass.ts
ds = bass.ds


def build_program(seq=S, mm_dt=BF16, n_cores=NC, phases=3,
                  collective=True, reps=1, agmode="chunk"):
    """Build the SPMD Bass/Tile program (identical on every core).

    collective=False replaces the AllGather with a same-shape local DMA
    (for single-core TimelineSim timing analysis only).
    reps>1 repeats the whole kernel body back-to-back inside one NEFF
    (for on-hardware marginal-time benchmarking).
    """
    nc = bacc.Bacc("TRN2", target_bir_lowering=False, debug=False,
                   num_devices=n_cores)
    T = B * seq
    nqc = seq // QC
    nkb = seq // KB
    ntb = T // KB

    xT_h = nc.declare_dram_parameter("xT", [D, T], mm_dt, isOutput=False)
    wqk_h = nc.declare_dram_parameter("wqkT", [128, 2, NDB, 128], mm_dt,
                                      isOutput=False)
    wv_h = nc.declare_dram_parameter("wvT", [128, NDB, 128], mm_dt, isOutput=False)
    wo_h = nc.declare_dram_parameter("woT", [128, NDB, 128], mm_dt, isOutput=False)
    ra_h = nc.declare_dram_parameter("ropeA", [128, seq], mm_dt, isOutput=False)
    rb_h = nc.declare_dram_parameter("ropeB", [128, seq], mm_dt, isOutput=False)
    sel_h = nc.declare_dram_parameter("selT", [HPC, 128],
                                      mybir.dt.float32r, isOutput=False)
    yT_h = nc.declare_dram_parameter("yT", [128, T], F32, isOutput=True)

    EXP = mybir.ActivationFunctionType.Exp
    scale = 1.0 / float(np.sqrt(DK))
    KPQ = QC // KB  # k-blocks per q-chunk

    with tile.TileContext(nc, num_cores=n_cores) as tc, ExitStack() as ctx:
        consts = ctx.enter_context(tc.tile_pool(name="consts", bufs=1))
        persist = ctx.enter_context(tc.tile_pool(name="persist", bufs=1))

        ra_t = consts.tile([128, seq], mm_dt)
        nc.sync.dma_start(ra_t[:], ra_h[:, :])
        rb_t = consts.tile([128, seq], mm_dt)
        nc.sync.dma_start(rb_t[:], rb_h[:, :])
        wqks = consts.tile([128, 2, NDB, 128], mm_dt)
        nc.sync.dma_start(wqks[:], wqk_h[:, :, :, :])
        wvs = consts.tile([128, NDB, 128], mm_dt)
        nc.sync.dma_start(wvs[:], wv_h[:, :, :])
        wos = consts.tile([128, NDB, 128], mm_dt)
        nc.sync.dma_start(wos[:], wo_h[:, :, :])
        sels = consts.tile([HPC, 128], mybir.dt.float32r)
        nc.sync.dma_start(sels[:], sel_h[:, :])
        triu = consts.tile([128, 128], mm_dt)
        make_upper_triangular(nc, triu[:], val=1.0, diag=True)
        ident = consts.tile([128, 128], mm_dt)
        make_identity(nc, ident[:])

        # QR/KR rows: [h0_r1(0:32), h0_r2(32:64), h1_r1(64:96), h1_r2(96:128)]
        QR = persist.tile([128, T], mm_dt)
        KR = persist.tile([128, T], mm_dt)
        # V in [token, feature] layout per 128-token block; per head a
        # 65-col block [v_h(0:64) | ones(64)] so P@V emits the softmax
        # denominator as out row 64 for either head
        VA = persist.tile([128, ntb, 130], mm_dt)
        # attention output slice, transposed: rows [h0 feats(0:64), h1(64:128)]
        ATT = persist.tile([128, T], mm_dt)

        # ---- Interleaved per-batch pipeline ------------------------------
        with (tc.tile_pool(name="ph1x", bufs=2) as xpool,
              tc.tile_pool(name="ph1s", bufs=2) as sbp,
              tc.tile_pool(name="dupt", bufs=2) as dpup,
              tc.tile_pool(name="ropet", bufs=2) as rpool,
              tc.tile_pool(name="sps", bufs=2, space="PSUM") as sps,
              tc.tile_pool(name="pvps", bufs=2, space="PSUM") as pvps,
              tc.tile_pool(name="auxps", bufs=2, space="PSUM") as aux,
              tc.tile_pool(name="ptp", bufs=18) as ptp,
              tc.tile_pool(name="nrm", bufs=2) as nrm,
              tc.tile_pool(name="dram", bufs=1, space="DRAM") as dpool,
              tc.tile_pool(name="agp", bufs=2) as agp,
              tc.tile_pool(name="ysb", bufs=3) as ysb):
            do_p3 = phases >= 3
            ag_space = "Shared" if collective else "Local"
            nc.vector.memset(VA[:, :, 64], 1.0)
            nc.vector.memset(VA[:, :, 129], 1.0)
            xr = xT_h[:, :].rearrange("(i p) t -> p i t", p=128)

            def all_gather(inp, outp):
                if collective:
                    nc.gpsimd.collective_compute(
                        "AllGather", mybir.AluOpType.bypass,
                        replica_groups=[list(range(n_cores))],
                        ins=[inp[:, :].opt()], outs=[outp[:, :].opt()])
                else:  # sim stand-in for the SDMA traffic (SP queue)
                    for cc in range(n_cores):
                        nc.sync.dma_start(
                            outp[ds(128 * cc, 128), :], inp[:, :])

            def wo_chunk(agc, bb, cc, whole=True):
                agr = agc[:, :].rearrange("(i p) t -> p i t", p=128)
                rt = agp.tile([128, NDB, QC], mm_dt, tag="rt", name="rt")
                nc.sync.dma_start(rt[:], agr[:, :, :] if whole
                                  else agr[:, :, ts(cc, QC)])
                yp = aux.tile([128, QC], F32, tag="aux", name="yp")
                for i in range(NDB):
                    nc.tensor.matmul(yp[:], wos[:, i, :], rt[:, i, :],
                                     start=(i == 0), stop=(i == NDB - 1))
                ysb_t = ysb.tile([128, QC], F32, tag="ys", name="ysb_t")
                nc.vector.tensor_copy(ysb_t[:], yp[:])
                nc.sync.dma_start(yT_h[:, ds(bb * seq + cc * QC, QC)],
                                  ysb_t[:])

            for rep in range(reps):
              # per-(batch, chunk) AllGather buffers: spreads collective +
              # HBM traffic through the batch (fresh set per rep: Shared
              # tiles allow only one writer). agmode="mixed" uses one
              # batch-level AG for b < B-1 (fewer collective floors) and
              # chunk AGs only for the last batch's tail pipeline.
              chunked = [agmode == "chunk" or _b == B - 1 for _b in range(B)]
              attcs, agcs = [], []
              for _b in range(B):
                if chunked[_b]:
                    attcs.append([dpool.tile([128, QC], mm_dt,
                                             name=f"attc{rep}_{_b}_{_q}")
                                  for _q in range(nqc)])
                    agcs.append([dpool.tile([128 * n_cores, QC], mm_dt,
                                            addr_space=ag_space,
                                            name=f"agc{rep}_{_b}_{_q}")
                                 for _q in range(nqc)])
                else:
                    attcs.append(dpool.tile([128, seq], mm_dt,
                                            name=f"attc{rep}_{_b}"))
                    agcs.append(dpool.tile([128 * n_cores, seq], mm_dt,
                                           addr_space=ag_space,
                                           name=f"agc{rep}_{_b}"))
              for b in range(B):
                # ---- projections + swap-DMA + rope + V for 4 chunks ----
                for ci in range(nqc):
                    c = b * nqc + ci
                    bsl = ds(ci * QC, QC)
                    xc = xpool.tile([128, NDB, QC], mm_dt, tag="xc")
                    # ACT's DGE queue: keeps batch-tail SP traffic (AG/rt/
                    # stand-ins) from delaying the next batch's x loads
                    nc.scalar.dma_start(xc[:], xr[:, :, ts(c, QC)])

                    xsb = []
                    for g in range(2):
                        ps = aux.tile([128, QC], F32, tag="aux", name="ps")
                        for i in range(NDB):
                            nc.tensor.matmul(ps[:], wqks[:, g, i, :],
                                             xc[:, i, :],
                                             start=(i == 0), stop=(i == NDB - 1))
                        xg = sbp.tile([128, QC], mm_dt, tag=f"xg{g}", name="xg")
                        nc.vector.tensor_copy(xg[:], ps[:])
                        xsb.append(xg)
                    # swap32: exchange adjacent 32-row pairs so x2 sits under
                    # x1's lanes (and vice versa); q on SP HWDGE, k on ACT
                    # HWDGE queue
                    sx = []
                    for g, eng in ((0, nc.sync), (1, nc.scalar)):
                        sxt = dpup.tile([128, QC], mm_dt, tag=f"sx{g}",
                                        name="sx")
                        for blk in range(2):
                            for du in range(2):
                                eng.dma_start(
                                    sxt[ds(64 * blk + 32 * du, 32), :],
                                    xsb[g][ds(64 * blk + 32 * (1 - du), 32), :])
                        sx.append(sxt)
                    # rope: QR/KR = x*C + swap32(x)*D  (q on DVE, k on Pool)
                    for g, OUT, eng in ((0, QR, nc.vector),
                                        (1, KR, nc.gpsimd)):
                        t1 = rpool.tile([128, QC], mm_dt, tag=f"t1{g}",
                                        name="t1")
                        t2 = rpool.tile([128, QC], mm_dt, tag=f"t2{g}",
                                        name="t2")
                        eng.tensor_mul(t1[:], xsb[g][:], ra_t[:, bsl])
                        eng.tensor_mul(t2[:], sx[g][:], rb_t[:, bsl])
                        eng.tensor_add(OUT[:, ts(c, QC)], t1[:], t2[:])

                    # V: project transposed then PE-transpose 128-blocks
                    psv = aux.tile([128, QC], F32, tag="aux", name="psv")
                    for i in range(NDB):
                        nc.tensor.matmul(psv[:], wvs[:, i, :], xc[:, i, :],
                                         start=(i == 0), stop=(i == NDB - 1))
                    vtc = sbp.tile([128, QC], mm_dt, tag="vtc", name="vtc")
                    nc.vector.tensor_copy(vtc[:], psv[:])
                    for sb in range(QC // 128):
                        tb = c * (QC // 128) + sb
                        tp = aux.tile([128, 128], mm_dt, tag="aux", name="tp")
                        nc.tensor.transpose(tp[:], vtc[:, ts(sb, 128)],
                                            ident[:])
                        vav = VA[:, tb, 0:130].rearrange(
                            "p (a c) -> p a c", a=2, c=65)[:, :, 0:64]
                        tpv = tp[:, :].rearrange("p (a c) -> p a c",
                                                 a=2, c=64)
                        nc.vector.tensor_copy(vav, tpv)

                last = b == B - 1
                # ---- attention for this batch (per q-chunk, two phases) --
                for qi in range(nqc):
                    qsl = ds(b * seq + qi * QC, QC)
                    nk = (qi + 1) * KPQ
                    # phase 1: scores (row-tiled head pair) + exp -> pt
                    pts = []
                    for kb in range(nk):
                        dj = kb - qi * KPQ
                        qv = 128 * dj if dj > 0 else 0
                        ksl = ds(b * seq + kb * KB, KB)
                        qslv = ds(b * seq + qi * QC + qv, QC - qv)
                        sp = sps.tile([128, 2, QC], F32, tag="sp", name="sp")
                        for hl in range(HPC):
                            nc.tensor.matmul(sp[:, hl, qv:QC],
                                             KR[ds(64 * hl, 64), ksl],
                                             QR[ds(64 * hl, 64), qslv],
                                             start=True, stop=True,
                                             tile_position=(64 * hl, 0))
                        pt = ptp.tile([128, 2, QC], mm_dt, tag="pt",
                                      name="pt")
                        nc.scalar.activation(pt[:, :, qv:QC], sp[:, :, qv:QC],
                                             EXP, scale=scale)
                        if dj >= 0:  # diagonal block: causal mask both heads
                            dsl = ds(128 * dj, 128)
                            for hl, eng in ((0, nc.vector), (1, nc.gpsimd)):
                                eng.tensor_mul(pt[:, hl, dsl],
                                               pt[:, hl, dsl],
                                               triu[:])
                        pts.append((pt, qv))
                    # phase 2: P@V accumulation per head + denominator row
                    dbt = nrm.tile([HPC, QC], F32, tag="dbt", name="dbt")
                    for hl in range(HPC):
                        pv = pvps.tile([65, QC], F32, tag="pv", name="pv")
                        for kb in range(nk):
                            pt, qv = pts[kb]
                            tbg = b * nkb + kb
                            nc.tensor.matmul(pv[:, qv:QC],
                                             VA[:, tbg, ds(65 * hl, 65)],
                                             pt[:, hl, qv:QC],
                                             start=(kb == 0),
                                             stop=(kb == nk - 1))
                        nc.vector.tensor_copy(ATT[ds(64 * hl, 64), qsl],
                                              pv[ds(0, 64), :])
                        deng = nrm.tile([65, QC], F32, tag="deng",
                                        name="deng")
                        nc.vector.tensor_copy(deng[ds(64, 1), :],
                                              pv[ds(64, 1), :])
                        nc.sync.dma_start(dbt[ds(hl, 1), :],
                                          deng[ds(64, 1), :])
                    # ---- per-chunk normalize ----
                    rbt = nrm.tile([HPC, QC], mybir.dt.float32r,
                                   tag="rbt", name="rbt")
                    with nc.allow_low_precision(reason="fp32r bcast rhs"):
                        nc.vector.reciprocal(rbt[:], dbt[:])
                    bp = aux.tile([128, QC], F32, tag="aux", name="bp")
                    nc.tensor.matmul(bp[:], sels[:, :], rbt[:],
                                     start=True, stop=True)
                    nc.vector.tensor_mul(ATT[:, qsl], ATT[:, qsl], bp[:])

                    if do_p3:
                        # AllGather; w_o chunk-deferred one batch
                        # (same-batch for the last batch's tail pipeline).
                        # Pool's queue stays collective-only: it blocks on
                        # completion waits, nothing else may queue there.
                        if chunked[b]:
                            nc.sync.dma_start(attcs[b][qi][:, :],
                                              ATT[:, qsl])
                        else:
                            nc.sync.dma_start(attcs[b][:, ts(qi, QC)],
                                              ATT[:, qsl])
                        if b > 0:
                            if chunked[b - 1]:
                                wo_chunk(agcs[b - 1][qi], b - 1, qi)
                            else:
                                wo_chunk(agcs[b - 1], b - 1, qi,
                                         whole=False)
                        if chunked[b]:
                            all_gather(attcs[b][qi], agcs[b][qi])
                            if last:
                                wo_chunk(agcs[b][qi], b, qi)
                        elif qi == nqc - 1:
                            all_gather(attcs[b], agcs[b])
            if not do_p3:
                nc.gpsimd.dma_start(yT_h[:, 0:seq], ra_t[:, :])

    nc.compile()
    return nc


def prep_inputs(inputs, seq=S, mm_dt=BF16, n_cores=NC):
    """Host-side sharding: build the per-core input maps."""
    mm_np = ml_dtypes.bfloat16 if mm_dt == BF16 else np.float32
    x = np.asarray(inputs["in_features"], dtype=np.float32)
    pos = np.asarray(inputs["token_positions"]).astype(np.float32)
    wq = np.asarray(inputs["w_q"], dtype=np.float32)
    wk = np.asarray(inputs["w_k"], dtype=np.float32)
    wv = np.asarray(inputs["w_v"], dtype=np.float32)
    wo = np.asarray(inputs["w_o"], dtype=np.float32)

    T = B * seq
    xT = np.ascontiguousarray(x.reshape(T, D).T).astype(mm_np)

    # rope tables (f32, matching reference numerics):
    #   QR = xq*C + swap32(xq)*D with xq rows [h0ev, h0od, h1ev, h1od];
    #   C = [cos]*4 ; D = [-sin, sin, -sin, sin]
    inv = np.float32(THETA) ** (-np.arange(0, DK, 2, dtype=np.float32)
                                / np.float32(DK))
    ang = pos[:, None].astype(np.float32) * inv[None, :].astype(np.float32)
    cosT = np.cos(ang.astype(np.float32)).T  # [32, seq]
    sinT = np.sin(ang.astype(np.float32)).T
    ropeA = np.ascontiguousarray(
        np.concatenate([cosT, cosT, cosT, cosT], axis=0)).astype(mm_np)
    ropeB = np.ascontiguousarray(
        np.concatenate([-sinT, sinT, -sinT, sinT], axis=0)).astype(mm_np)

    # normalize broadcast selectors: out row m takes denominator row m // 64
    selT = np.zeros((HPC, 128), dtype=np.float32)
    for m in range(128):
        selT[m // 64, m] = 1.0

    ev = np.arange(0, DK, 2)
    od = ev + 1

    def lhsT_stack(W):
        # W [128 out-features, D] -> [128, NDB, 128]; [:, i, :] = W[:, 128i:+128].T
        Wt = np.ascontiguousarray(W.T).astype(mm_np)  # [D, 128]
        return np.ascontiguousarray(
            Wt.reshape(NDB, 128, 128).transpose(1, 0, 2))

    in_maps = []
    for c in range(n_cores):
        h0, h1 = HPC * c, HPC * c + 1
        W1 = np.concatenate([wq[DK * h0 + ev], wq[DK * h0 + od],
                             wq[DK * h1 + ev], wq[DK * h1 + od]], axis=0)
        W2 = np.concatenate([wk[DK * h0 + ev], wk[DK * h0 + od],
                             wk[DK * h1 + ev], wk[DK * h1 + od]], axis=0)
        wqkT = np.ascontiguousarray(np.stack(
            [lhsT_stack(Wg) for Wg in (W1, W2)], axis=1))
        WV = wv[128 * c: 128 * (c + 1)]
        WO = wo[128 * c: 128 * (c + 1)]
        in_maps.append({
            "xT": xT,
            "wqkT": wqkT,
            "wvT": lhsT_stack(WV),
            "woT": lhsT_stack(WO),
            "ropeA": ropeA,
            "ropeB": ropeB,
            "selT": selT,
        })
    return in_maps


def assemble_output(results, seq=S, n_cores=NC):
    yT = np.concatenate([np.asarray(r["yT"], dtype=np.float32)
                         for r in results], axis=0)  # [1024, T]
    return np.ascontiguousarray(yT.T).reshape(B, seq, D).astype(np.float32)


_PROGRAM_CACHE = {}


def kernel(**inputs) -> np.ndarray:
    key = ("full", S, "bf16")
    if key not in _PROGRAM_CACHE:
        _PROGRAM_CACHE[key] = build_program(seq=S, mm_dt=BF16, n_cores=NC)
    nc = _PROGRAM_CACHE[key]
    in_maps = prep_inputs(inputs, seq=S, mm_dt=BF16, n_cores=NC)
    res = run_bass_kernel_spmd(nc, in_maps, list(range(NC)))
    return assemble_output(res.results)


if __name__ == "__main__":
    from ref_np import make_inputs
    inputs = make_inputs(seed=0, S=S)
    out = kernel(**inputs)
    print(out.shape, out.dtype)
